# revision 11
# baseline (speedup 1.0000x reference)
"""Trainium2 Bass kernel for the PlaneElement kinematic-wave step.

Contract: kernel(**inputs) takes the FULL (unsharded) numpy inputs and
returns the full output -- 4 scalars:
    (outflow_q, infil_rate_element, infil_depth_element, max_cfl)
as a float32 array of shape (4,).

Strategy:
  - Shard the 4M-node axis contiguously across 8 NeuronCores.
  - Each core gets a [128, 4099] f32 tile: partition p holds its 4096
    owned nodes plus a 3-element stencil halo (2 left, 1 right), baked
    in on the host (as_strided) -> no device halo exchange.
  - On-device math (per core, in "SF = A/WID" units), chunked along the
    free dim for DVE/ACT/DMA pipelining:
      SF      = relu(alpha*d + beta)     [ScalarE, fused sum accum]
      sum(d)                              [ScalarE copy, fused accum]
      minmod slope via clamp identity     [DVE]
      SFface  = SF + 0.5*minmod           [DVE STT]
      flux'   = SFface * exp(2/3*(lnAs-lnwp) + ln(r*m))  [ScalarE+DVE]
      SFnext  = relu(SF - dflux')         [ScalarE]
      g2      = lnAs2 - lnwp2, reduce max [DVE]
    max(vel) = m*exp(2/3*max g2) recovered on host (exp is monotone);
    sum(infil) = sum(d) + C*b0 - sum(SF) (exact identity; halo terms
    subtracted on host).
  - Tiny per-core partials are combined on the host; the outlet
    discharge and the two inlet-boundary nodes are computed exactly on
    the host in f64.
"""

import math

import numpy as np

N = 4_194_304
EPS = 1e-8
NCORES = 8
P = 128
F = 4096          # owned elements per partition
C = P * F         # owned elements per core
W = F + 3         # tile width incl. 3 halo columns
NCH = 4           # free-dim chunks for pipelining
CF = F // NCH     # owned columns per chunk

_prog_cache = {}


def _manning_q_np(A, WID, SS1, SS2, MAN, SL):
    h = A / WID
    wp = WID + h * (math.sqrt(1.0 + SS1 * SS1) + math.sqrt(1.0 + SS2 * SS2))
    A_safe = max(A, EPS)
    return A * (A_safe / wp) ** (2.0 / 3.0) * math.sqrt(SL) / MAN


def _build_program(consts, nch):
    import concourse.bacc as bacc
    import concourse.mybir as mybir
    from concourse.tile import TileContext

    (alpha, beta, b0, sconst, wid, ln_rm) = consts
    cf = F // nch
    outc = 4 * nch + 1
    f32 = mybir.dt.float32
    Alu = mybir.AluOpType
    Act = mybir.ActivationFunctionType
    X = mybir.AxisListType.X

    nc = bacc.Bacc("TRN2", target_bir_lowering=False, debug=False,
                   num_devices=NCORES)
    d_in = nc.dram_tensor("d", [P, W], f32, kind="ExternalInput")
    o_out = nc.dram_tensor("out", [P, outc], f32, kind="ExternalOutput")

    # activation-bias constants must exist as const APs
    for i, val in enumerate({float(beta), float(EPS), float(wid),
                             float(ln_rm)}):
        if (f32, val) in nc.const_aps.aps:
            continue
        ct = nc.alloc_sbuf_tensor(f"constb-{i}", [P, 1], f32)
        nc.gpsimd.memset(ct.ap(), val)
        nc.const_aps.aps[(f32, val)] = ct.ap()
    nc.all_engine_barrier()

    with TileContext(nc) as tc:
        with tc.tile_pool(name="pool", bufs=2) as pool:
            out_tile = pool.tile([P, outc], f32, bufs=1)
            for c in range(nch):
                o = c * cf
                L = cf + 3

                dd = pool.tile([P, L], f32)
                nc.sync.dma_start(out=dd[:], in_=d_in[:, o:o + L])

                # surface depth; fused window sums of SF and d
                SF = pool.tile([P, L], f32)
                nc.scalar.activation(SF[:], dd[:], Act.Relu,
                                     bias=beta, scale=alpha,
                                     accum_out=out_tile[:, c:c + 1])
                dsc = pool.tile([P, L], f32)
                nc.scalar.activation(dsc[:], dd[:], Act.Copy,
                                     accum_out=out_tile[:, nch + c:
                                                        nch + c + 1])

                # MUSCL limiter: minmod(x,y) = clamp(y, min(x,0), max(x,0))
                dSF = pool.tile([P, L - 1], f32)
                nc.vector.tensor_sub(dSF[:], SF[:, 1:L], SF[:, 0:L - 1])
                xm = pool.tile([P, L - 1], f32)
                nc.vector.tensor_scalar_min(xm[:], dSF[:], 0.0)
                xp = pool.tile([P, L - 1], f32)
                nc.vector.tensor_scalar_max(xp[:], dSF[:], 0.0)
                c1 = pool.tile([P, L - 2], f32)
                nc.vector.tensor_tensor(c1[:], dSF[:, 1:L - 1],
                                        xm[:, 0:L - 2], Alu.max)
                c2 = pool.tile([P, L - 2], f32)
                nc.vector.tensor_tensor(c2[:], c1[:], xp[:, 0:L - 2], Alu.min)
                SFf = pool.tile([P, L - 2], f32)
                nc.vector.scalar_tensor_tensor(
                    SFf[:], c2[:], 0.5, SF[:, 1:L - 1], Alu.mult, Alu.add)

                # Manning flux on face states, in log space
                lnAs = pool.tile([P, L - 2], f32)
                nc.scalar.activation(lnAs[:], SFf[:], Act.Ln,
                                     bias=EPS, scale=wid)
                lnwp = pool.tile([P, L - 2], f32)
                nc.scalar.activation(lnwp[:], SFf[:], Act.Ln,
                                     bias=wid, scale=sconst)
                g1 = pool.tile([P, L - 2], f32)
                nc.vector.tensor_sub(g1[:], lnAs[:], lnwp[:])
                pw = pool.tile([P, L - 2], f32)
                nc.scalar.activation(pw[:], g1[:], Act.Exp,
                                     bias=ln_rm, scale=2.0 / 3.0)
                fx = pool.tile([P, L - 2], f32)
                nc.vector.tensor_mul(fx[:], SFf[:], pw[:])

                # conservative update
                fd = pool.tile([P, cf], f32)
                nc.vector.tensor_sub(fd[:], fx[:, 1:cf + 1], fx[:, 0:cf])
                s2 = pool.tile([P, cf], f32)
                nc.vector.tensor_sub(s2[:], SF[:, 2:2 + cf], fd[:])
                SFn = pool.tile([P, cf], f32)
                nc.scalar.activation(SFn[:], s2[:], Act.Relu)

                # CFL: g2 = ln(A_safe) - ln(wp) on updated state, reduce max
                lnA2 = pool.tile([P, cf], f32)
                nc.scalar.activation(lnA2[:], SFn[:], Act.Ln,
                                     bias=EPS, scale=wid)
                lnw2 = pool.tile([P, cf], f32)
                nc.scalar.activation(lnw2[:], SFn[:], Act.Ln,
                                     bias=wid, scale=sconst)
                g2 = pool.tile([P, cf], f32)
                nc.vector.tensor_sub(g2[:], lnA2[:], lnw2[:])
                nc.vector.tensor_reduce(
                    out_tile[:, 2 * nch + c:2 * nch + c + 1], g2[:, 2:cf],
                    X, Alu.max)
                nc.vector.tensor_reduce(
                    out_tile[:, 3 * nch + c:3 * nch + c + 1], g2[:, 0:2],
                    X, Alu.max)

                if c == nch - 1:
                    nc.vector.tensor_copy(out_tile[:, 4 * nch:4 * nch + 1],
                                          SFn[:, cf - 1:cf])

            nc.sync.dma_start(out=o_out[:, :], in_=out_tile[:])

    nc.compile()
    return nc


def _run_device(shards, consts, nch, trace=False):
    from concourse.bass_utils import run_bass_kernel_spmd

    key = (tuple(consts), nch)
    if key not in _prog_cache:
        _prog_cache[key] = _build_program(consts, nch)
    nc = _prog_cache[key]
    in_maps = [{"d": shards[i]} for i in range(NCORES)]
    res = run_bass_kernel_spmd(nc, in_maps, core_ids=list(range(NCORES)),
                               trace=trace)
    return res


def kernel(depth, rain_rate, dt, cum_rain, theta_current, F_cumulative,
           WID, SS1, SS2, MAN, SL, dx, Ks, psi, theta_s, _trace=False,
           _return_results=False, _nch=NCH):
    depth = np.asarray(depth, np.float32)
    rain_rate = float(rain_rate)
    dt = float(dt)
    theta_current = float(theta_current)
    F_cumulative = float(F_cumulative)
    WID = float(WID)
    SS1 = float(SS1)
    SS2 = float(SS2)
    MAN = float(MAN)
    SL = float(SL)
    dx = float(dx)
    Ks = float(Ks)
    psi = float(psi)
    theta_s = float(theta_s)

    # host-folded scalar coefficients (f64)
    dtheta = max(theta_s - theta_current, 0.0)
    F_safe = max(F_cumulative, 1e-6)
    a1 = Ks * dt / F_safe                       # fp*dt = a0 + a1*d
    a0 = Ks * dt * (1.0 + psi * dtheta / F_safe)
    b0 = rain_rate * dt                         # avail = d + b0
    alpha = 1.0 - a1                            # surf = relu(alpha*d + beta)
    beta = b0 - a0
    sconst = math.sqrt(1.0 + SS1 * SS1) + math.sqrt(1.0 + SS2 * SS2)
    m = math.sqrt(SL) / MAN
    r = dt / dx
    # In SF = A/WID units: SF_next = relu(SF - (f_i - f_{i-1})) with
    #   f = (r/WID)*q(A_face) = r*m*SFface*ratio^(2/3),
    #   ratio = max(WID*SFface, EPS)/(WID + sconst*SFface)
    # lnAs = ln(WID*SFface + EPS), lnwp = ln(WID + sconst*SFface),
    # exp bias = ln(r*m).
    ln_rm = math.log(max(r * m, 1e-38))
    consts = (alpha, beta, b0, sconst, WID, ln_rm)

    # --- host shard prep: [128, 4099] per core with baked halo ---
    padded = np.empty(N + 3, np.float32)
    padded[2:2 + N] = depth
    padded[0:2] = 0.0          # left ghosts (nodes 0,1 host-corrected)
    padded[N + 2] = depth[-1]  # right ghost replicates -> slope[N-1] = 0
    shards = []
    for k in range(NCORES):
        base = padded[k * C:k * C + C + 3]
        sh = np.lib.stride_tricks.as_strided(
            base, shape=(P, W), strides=(F * 4, 4)).copy()
        shards.append(np.ascontiguousarray(sh))

    res = _run_device(shards, consts, _nch, trace=_trace)
    outs = [res.results[i]["out"] for i in range(NCORES)]

    nch = _nch
    cf = F // nch

    # --- host combine ---
    # halo columns per chunk: local j in {o, o+1, o+cf+2}
    halo_j = np.concatenate([[c * cf, c * cf + 1, c * cf + cf + 2]
                             for c in range(nch)])
    sum_SF = np.float64(0.0)
    sum_d = np.float64(0.0)
    for k in range(NCORES):
        sum_SF += np.sum(outs[k][:, 0:nch].astype(np.float64))
        sum_d += np.sum(outs[k][:, nch:2 * nch].astype(np.float64))
        dh = shards[k][:, halo_j].astype(np.float64)
        sum_d -= dh.sum()
        sum_SF -= np.maximum(alpha * dh + beta, 0.0).sum()
    sum_t = sum_d + N * b0 - sum_SF
    infil_depth = sum_t / N
    infil_rate = infil_depth / dt

    g2max = -np.inf
    for k in range(NCORES):
        g2max = max(g2max, float(outs[k][:, 2 * nch:3 * nch].max()))
        edge = outs[k][:, 3 * nch:4 * nch].astype(np.float64).copy()
        if k == 0:
            edge[0, 0] = -np.inf  # polluted inlet nodes 0,1
        g2max = max(g2max, float(edge.max()))
    max_vel = m * math.exp((2.0 / 3.0) * g2max) if m > 0 else 0.0

    # exact inlet nodes 0 and 1 on host (f64), matching reference BCs
    d0, d1, d2 = (float(depth[0]), float(depth[1]), float(depth[2]))

    def _surf(d):
        t = min(a0 + a1 * d, d + b0)
        return max(d + b0 - t, 0.0)

    A0, A1, A2 = (WID * _surf(d0), WID * _surf(d1), WID * _surf(d2))
    # slope[0] = 0; slope[1] = minmod(A1-A0, A2-A1)
    x, y = A1 - A0, A2 - A1
    mm1 = min(max(y, min(x, 0.0)), max(x, 0.0))
    Af0 = A0
    Af1 = A1 + 0.5 * mm1
    q0 = _manning_q_np(Af0, WID, SS1, SS2, MAN, SL)
    q1 = _manning_q_np(Af1, WID, SS1, SS2, MAN, SL)
    An0 = max(A0 - r * (q0 - 0.0), 0.0)
    An1 = max(A1 - r * (q1 - q0), 0.0)
    for An in (An0, An1):
        Q = _manning_q_np(An, WID, SS1, SS2, MAN, SL)
        max_vel = max(max_vel, Q / max(An, EPS))

    max_cfl = max_vel * dt / dx

    # outlet discharge from the device's last updated state
    sfl = float(outs[NCORES - 1][P - 1, 4 * nch])
    A_last = WID * sfl
    outflow_q = _manning_q_np(A_last, WID, SS1, SS2, MAN, SL)

    out = np.array([outflow_q, infil_rate, infil_depth, max_cfl], np.float32)
    if _return_results:
        return out, res
    return out
